# revision 24
# baseline (speedup 1.0000x reference)
"""Trainium2 Bass kernel for MedicalAttentionLayer (B=2, S=2048, D=1024, H=16).

Sharding (8 NeuronCores): core c = (b, g) with b = c // 4, g = c % 4; core
(b, g) owns tokens [512g, 512g+512) of batch b.
 - Q/K/V projections are token-sharded (each core projects its own 512
   tokens, all 16 heads).
 - K^T and V shards are AllGather'd within each 4-core batch group.
 - Attention runs on own 512 query tokens x all 16 heads x all 2048 keys.
 - Output projection + residual + layernorm are local (token-sharded).

Precision/scaling scheme (fp8e4m3 DoubleRow matmuls at 2x PE rate):
 - X^T unscaled fp8; all weights x32 in fp8 (W ~0.02 needs the scale to
   stay in fp8's normal range).
 - Q^T/K^T kept bf16 at x32 (bias add from a pre-scaled bias); scores
   PSUM carries x1024, folded into the exp scale (1/8192 instead of 1/8).
 - V' carries x32 in fp8 with the softmax-denominator ones column at
   32.0, so the x32 cancels in the normalize (num/den). The V bias is
   accumulated in PSUM by an extra K=1 matmul (ones-column x bias-row).
 - ctx normalized to scale 1, cast fp8; out-proj = nctx (32 Wo) carries
   x32 into the residual, with xres pre-scaled x32 host-side; LayerNorm
   is scale-invariant so no dequant is ever needed.
 - DoubleRow layout: contraction index k = j*128 + p -> tile [128 p,
   nsub j, free]; each matmul consumes a [:, 2t:2t+2, :] slice.

Scores (q.k, contraction 64) remain bf16: the critical engine is the
scalar-engine exp over S^2 H / 8 = 16.8M elements (~110us floor), so PE
work below that bound is free. The schedule keeps the exp backbone
saturated: K-proj starts as soon as its weights land (DMA priority
order), Q-proj m=1..7 chunks are interleaved into the attention loop,
V'-projection/gather overlaps the first heads (es pool buffers 8
groups), and the tail LayerNorm computes moments via scalar-engine
accum_out with the gamma multiply on gpsimd.
"""

import numpy as np
import ml_dtypes

# ---- problem constants (hardcoded; kernel.py must be self-contained) ----
B = 2
S = 2048
D = 1024
H = 16
DH = 64
LN_EPS = 1e-5
NCORES = 8
G = 4                 # cores per batch group
TPC = S // G          # tokens per core = 512
SCALE = 1.0 / 8.0     # 1/sqrt(DH)
WS = 32.0             # fp8 weight scale
ESCALE = SCALE / (WS * WS)   # exp scale absorbing Q,K x32
VC = DH + 1           # V columns per head incl. ones column
NKB = S // 128        # 16 key blocks
NKP = NKB // 2        # 8 key-block pairs (DoubleRow ctx)
NTB = TPC // 128      # 4 own-token blocks

BF16 = ml_dtypes.bfloat16
FP8 = ml_dtypes.float8_e4m3
L2E = 12102203.161561485          # 2^23 / ln 2  (Schraudolph fast exp)
FEXP_BIAS = 1064866805.0          # (127 - 0.0579848) * 2^23

_CACHE = {}


def _build(reps=1, fake_ag=False):
    """Build the (single) SPMD Bass program. Returns the Bacc object."""
    from concourse import bacc, mybir, tile

    BF = mybir.dt.bfloat16
    F32 = mybir.dt.float32
    F8 = mybir.dt.float8e4
    I32 = mybir.dt.int32
    AF = mybir.ActivationFunctionType
    ALU = mybir.AluOpType
    DR = mybir.MatmulPerfMode.DoubleRow

    nc = bacc.Bacc("TRN2", target_bir_lowering=False, debug=False,
                   num_devices=NCORES)

    # ---------------- I/O ----------------
    # X^T in DoubleRow layout [128, 8, 512] flattened (k = j*128+p over d)
    xt8 = nc.dram_tensor("xt8", [128, 8 * TPC], F8, kind="ExternalInput")
    xres = nc.dram_tensor("xres", [TPC, D], F32, kind="ExternalInput")  # x32
    # weights x32, DoubleRow layout [128, 8, 1024] flattened
    wq8 = nc.dram_tensor("wq8", [128, 8 * D], F8, kind="ExternalInput")
    wk8 = nc.dram_tensor("wk8", [128, 8 * D], F8, kind="ExternalInput")
    wv8 = nc.dram_tensor("wv8", [128, 8 * D], F8, kind="ExternalInput")
    wo8 = nc.dram_tensor("wo8", [128, 8 * D], F8, kind="ExternalInput")
    bqv = nc.dram_tensor("bqv", [D], F32, kind="ExternalInput")  # x32
    bkv = nc.dram_tensor("bkv", [D], F32, kind="ExternalInput")  # x32
    bvrow = nc.dram_tensor("bvrow", [1, D], F8, kind="ExternalInput")  # x32
    medv = nc.dram_tensor("medv", [H], F32, kind="ExternalInput")
    # exp(mask) for own tokens, [128, NTB] column layout
    expm = nc.dram_tensor("expm", [128, NTB], F32, kind="ExternalInput")
    gamma = nc.dram_tensor("gamma", [D], F32, kind="ExternalInput")
    beta = nc.dram_tensor("beta", [D], F32, kind="ExternalInput")
    out = nc.dram_tensor("out", [TPC, D], F32, kind="ExternalOutput")

    RG = [[0, 1, 2, 3], [4, 5, 6, 7]]

    with tile.TileContext(nc) as tc:
        with (
            tc.tile_pool(name="persist", bufs=1) as pp,
            tc.tile_pool(name="dram", bufs=2, space="DRAM") as dp,
        ):
            # ---- DMA priority order: smalls, then wk8/xt8 (K-proj gate) ----
            w_t = {}
            for name in ("k", "q", "v", "o"):
                w_t[name] = pp.tile([128, 8 * D], F8, tag=f"w{name}",
                                    name=f"w{name}")
            b_t = {}
            for name, hndl in (("q", bqv), ("k", bkv)):
                t = pp.tile([128, 8], F32, tag=f"b{name}", name=f"b{name}")
                nc.scalar.dma_start(
                    t[:], hndl.ap().rearrange("(m p) -> p m", p=128))
                b_t[name] = t
            bvr_t = pp.tile([1, D], F8, tag="bvr", name="bvr")
            nc.scalar.dma_start(bvr_t[:], bvrow[:, :])
            ones1 = pp.tile([1, 128], F8, tag="ones1", name="ones1")
            nc.vector.memset(ones1[:], 1.0)
            med_t = pp.tile([128, H], F32, tag="med", name="med")
            nc.scalar.dma_start(med_t[:], medv[None, :].to_broadcast((128, H)))
            # fast-exp bits bias per head: FEXP_BIAS + L2E * med[h]
            fxb_t = pp.tile([128, H], F32, tag="fxb", name="fxb")
            nc.vector.tensor_scalar(fxb_t[:], med_t[:], L2E, FEXP_BIAS,
                                    ALU.mult, ALU.add)
            expm_t = pp.tile([128, NTB], F32, tag="expm", name="expm")
            nc.scalar.dma_start(expm_t[:], expm[:, :])
            em8_t = pp.tile([128, NTB], F8, tag="em8", name="em8")
            nc.vector.tensor_scalar_mul(em8_t[:], expm_t[:], WS)

            nc.sync.dma_start(w_t["k"][:], wk8[:, :])
            xt_t = pp.tile([128, 8 * TPC], F8, tag="xt8", name="xt8")
            nc.scalar.dma_start(xt_t[:, 0:4 * TPC], xt8[:, 0:4 * TPC])
            nc.sync.dma_start(xt_t[:, 4 * TPC:], xt8[:, 4 * TPC:])
            nc.scalar.dma_start(w_t["q"][:], wq8[:, :])
            nc.sync.dma_start(w_t["v"][:], wv8[:, :])
            eps_t = pp.tile([128, 1], F32, tag="eps")
            nc.vector.memset(eps_t[:], LN_EPS)
            warm_t = pp.tile([128, 1], F32, tag="warm", name="warm")
            nc.scalar.activation(warm_t[:], eps_t[:], AF.Exp)

            # persistent local intermediates
            qt_t = [pp.tile([128, TPC], F8, tag=f"qt{i}", name=f"qt{i}")
                    for i in range(8)]

            # residual (x32) preloaded during phase 2
            xr_t = [pp.tile([128, D], F32, tag=f"xr{i}", name=f"xr{i}")
                    for i in range(NTB)]
            gb_t = {}
            for name in ("gamma", "beta"):
                gb_t[name] = pp.tile([128, D], F32, tag=name,
                                     name=f"gb_{name}")

            def xt_ap():
                return xt_t[:].rearrange("p (j n) -> p j n", n=TPC)

            def w_ap(name):
                return w_t[name][:].rearrange("p (j m) -> p j m", m=D)

            psmall_cm = tc.tile_pool(name="psmall", bufs=2, space="PSUM")
            psmall = psmall_cm.__enter__()
            pools_cm = [
        tc.tile_pool(name="kv_loc", bufs=3),
        tc.tile_pool(name="kvf", bufs=1),
        tc.tile_pool(name="scA", bufs=1, space="PSUM"),
        tc.tile_pool(name="scB", bufs=1, space="PSUM"),
        tc.tile_pool(name="es", bufs=10),
        tc.tile_pool(name="norm", bufs=2),
        tc.tile_pool(name="ln", bufs=2),
    ]
            kvp, kvf, scA, scB, esp, normp, lnp = [
        cm.__enter__() for cm in pools_cm]
            sc_pools = [scA, scB]

            ktf_t = [kvf.tile([128, S], F8, tag=f"ktf{m}", name=f"ktf{m}")
                     for m in range(8)]

            def emit_qchunk(m):
                ps = psmall.tile([128, TPC], F32, tag="ps", name="projq_ps")
                for t2 in range(4):
                    nc.tensor.matmul(
                        ps[:],
                        w_ap("q")[:, 2 * t2:2 * t2 + 2,
                                  m * 128:(m + 1) * 128],
                        xt_ap()[:, 2 * t2:2 * t2 + 2, :],
                        start=(t2 == 0), stop=(t2 == 3), perf_mode=DR)
                nc.vector.tensor_scalar_add(qt_t[m][:], ps[:],
                                            b_t["q"][:, m:m + 1])

            def phase1_steps(rep):
                """Closures emitting next-rep projections + gathers; rep's
                shard buffers are captured per call (dram pool bufs=2)."""
                kt_sh = dp.tile([D, TPC], F8, name=f"kt_sh{rep}")
                v_sh = dp.tile([TPC, H * DH + 1], F8, name=f"v_sh{rep}")
                kt_ag = dp.tile([G, D, TPC], F8, name=f"kt_ag{rep}")
                v_ag = dp.tile([G, TPC, H * DH + 1], F8, name=f"v_ag{rep}")

                def k_chunks(ms):
                    for m in ms:
                        ps = psmall.tile([128, TPC], F32, tag="ps",
                                         name="proj_ps")
                        for t2 in range(4):
                            nc.tensor.matmul(
                                ps[:],
                                w_ap("k")[:, 2 * t2:2 * t2 + 2,
                                          m * 128:(m + 1) * 128],
                                xt_ap()[:, 2 * t2:2 * t2 + 2, :],
                                start=(t2 == 0), stop=(t2 == 3),
                                perf_mode=DR)
                        ktl = kvp.tile([128, TPC], F8, tag="ktl",
                                       name=f"ktl{m}")
                        nc.vector.tensor_scalar_add(ktl[:], ps[:],
                                                    b_t["k"][:, m:m + 1])
                        nc.sync.dma_start(kt_sh[m * 128:(m + 1) * 128, :],
                                          ktl[:])

                def ag_k():
                    if not fake_ag:
                        nc.gpsimd.collective_compute(
                            "AllGather", mybir.AluOpType.bypass,
                            replica_groups=RG,
                            ins=[kt_sh[:].opt()], outs=[kt_ag[:].opt()])

                def v_block(tb):
                    vt = kvp.tile([128, H * DH], F8, tag="vl",
                                  name=f"vl{tb}")
                    for j in range(2):
                        pv = psmall.tile([128, 512], F32, tag="ps",
                                         name=f"projv_ps{j}")
                        for t2 in range(4):
                            nc.tensor.matmul(
                                pv[:],
                                xt_ap()[:, 2 * t2:2 * t2 + 2,
                                        tb * 128:(tb + 1) * 128],
                                w_ap("v")[:, 2 * t2:2 * t2 + 2,
                                          j * 512:(j + 1) * 512],
                                start=(t2 == 0), stop=False, perf_mode=DR)
                        # bias via ones-column x bias-row (K=1)
                        nc.tensor.matmul(
                            pv[:], ones1[:, :],
                            bvr_t[:, j * 512:(j + 1) * 512],
                            start=False, stop=True, skip_group_check=True)
                        # scale by exp(mask) while copying out of PSUM
                        nc.vector.tensor_scalar_mul(
                            vt[:, j * 512:(j + 1) * 512], pv[:],
                            expm_t[:, tb:tb + 1])
                    nc.sync.dma_start(
                        v_sh[tb * 128:(tb + 1) * 128, 0:H * DH], vt[:])
                    # denominator weight column: 32*exp(mask), gathered
                    # alongside V' as column H*DH
                    nc.sync.dma_start(
                        v_sh[tb * 128:(tb + 1) * 128, H * DH:H * DH + 1],
                        em8_t[:, tb:tb + 1])

                def ag_v():
                    if not fake_ag:
                        nc.gpsimd.collective_compute(
                            "AllGather", mybir.AluOpType.bypass,
                            replica_groups=RG,
                            ins=[v_sh[:].opt()], outs=[v_ag[:].opt()])

                def load_ktf(ms):
                    for m in ms:
                        if fake_ag:
                            src_ap = kt_sh[m * 128:(m + 1) * 128, None, :] \
                                .to_broadcast((128, G, TPC))
                        else:
                            src_ap = kt_ag[:, m * 128:(m + 1) * 128, :] \
                                .rearrange("g p t -> p g t")
                        nc.sync.dma_start(ktf_t[m][:], src_ap)

                vf_rep = []

                def load_vf(kps):
                    for kp in kps:
                        t = kvf.tile([128, 2 * H * DH], F8, tag=f"vf{kp}",
                                     name=f"vf{kp}", bufs=2)
                        em = kvf.tile([128, 2 * 32], F8, tag=f"em{kp}",
                                      name=f"em{kp}", bufs=2)
                        nc.gpsimd.memset(em[:], 0.0)
                        if fake_ag:
                            src_ap = v_sh[:].rearrange(
                                "(j p) c -> p j c", j=4)[:, 0:2, 0:H * DH]
                            em_src = v_sh[:].rearrange(
                                "(j p) c -> p j c", j=4) \
                                [:, 0:2, H * DH:H * DH + 1]
                        else:
                            full = v_ag[:].rearrange(
                                "g (i j p) c -> p (g i) j c", j=2, p=128) \
                                [:, kp, :, :]
                            src_ap = full[:, :, 0:H * DH]
                            em_src = full[:, :, H * DH:H * DH + 1]
                        nc.sync.dma_start(
                            t[:].rearrange("p (j c) -> p j c", c=H * DH),
                            src_ap)
                        nc.sync.dma_start(
                            em[:].rearrange("p (j c) -> p j c", c=32)
                            [:, :, 0:1], em_src)
                        vf_rep.append((t, em))

                steps = [
                    lambda: k_chunks([0, 1]),
                    lambda: k_chunks([2, 3]),
                    lambda: k_chunks([4, 5]),
                    lambda: k_chunks([6, 7]),
                    ag_k,
                    lambda: v_block(0),
                    lambda: v_block(1),
                    lambda: v_block(2),
                    lambda: v_block(3),
                    ag_v,
                    lambda: emit_qchunk(0),
                    lambda: load_ktf([0]),
                    lambda: load_vf([0, 1, 2, 3]),
                    lambda: load_vf([4, 5, 6, 7]),
                    lambda: load_ktf([1, 2, 3]),
                    lambda: load_ktf([4, 5, 6, 7]),
                ]
                return steps, vf_rep

            # insertion points (flat-loop iteration idx) for next-rep steps
            INS = [80, 84, 88, 92, 96, 98, 100, 102, 104, 106,
                   108, 110, 112, 114, 116, 120]

            # prologue: rep 0 phase 1
            steps0, vf0 = phase1_steps(0)
            for st in steps0:
                st()
            vf_cur = vf0
            tail_pending = None

            for rep in range(reps):
                if rep == 0:
                    for tb in range(NTB):
                        nc.sync.dma_start(
                            xr_t[tb][:], xres[tb * 128:(tb + 1) * 128, :])
                    nc.gpsimd.dma_start(w_t["o"][:], wo8[:, :])
                    nc.sync.dma_start(
                        gb_t["gamma"][:],
                        gamma[None, :].to_broadcast((128, D)))
                    nc.sync.dma_start(
                        gb_t["beta"][:],
                        beta[None, :].to_broadcast((128, D)))
                    # prefill output rows with beta; final stores accum
                    for tb in range(NTB):
                        nc.gpsimd.dma_start(
                            out[tb * 128:(tb + 1) * 128, :],
                            gb_t["beta"][:])

                # ---------------- phase 2: attention ----------------
                next_steps = None
                if rep + 1 < reps:
                    next_steps, vf_next = phase1_steps(rep + 1)

                # nctx double-buffered across reps: rep r+1's normalizes
                # write the other buffer while rep r's out-proj drains
                nctx_t = [kvf.tile([128, 2 * TPC], F8, tag=f"nctx{i}",
                                   name=f"nctx{i}", bufs=2)
                          for i in range(4)]
                flat = [(h, kp) for h in range(H) for kp in range(NKP)]
                ctx_tiles = {}
                vf_t = vf_cur

                def emit_ctx(h, kp, es):
                    if h not in ctx_tiles:
                        ctx_tiles[h] = (
                            psmall.tile([DH, TPC], F32, tag="ps",
                                        name="ctx"),
                            psmall.tile([32, TPC], F32, tag="den",
                                        name="den"))
                    ctx, den = ctx_tiles[h]
                    vt, em = vf_t[kp]
                    es_ap = es[:].rearrange("p (j n) -> p j n", n=TPC)
                    nc.tensor.matmul(
                        ctx[:],
                        vt[:].rearrange("p (j c) -> p j c", c=H * DH)
                        [:, :, h * DH:h * DH + DH],
                        es_ap,
                        start=(kp == 0), stop=(kp == NKP - 1),
                        perf_mode=DR)
                    nc.tensor.matmul(
                        den[:],
                        em[:].rearrange("p (j c) -> p j c", c=32),
                        es_ap,
                        start=(kp == 0), stop=(kp == NKP - 1),
                        perf_mode=DR, skip_group_check=True)

                def emit_norm(h):
                    ctx, den = ctx_tiles.pop(h)
                    rec = normp.tile([1, TPC], F32, tag="rec", name="rec")
                    nc.vector.reciprocal(rec[:], den[0:1, :])
                    rbc = normp.tile([DH, TPC], F32, tag="rbc", name="rbc")
                    nc.gpsimd.partition_broadcast(rbc[:], rec[:])
                    dst = nctx_t[h // 4][:].rearrange(
                        "p (j n) -> p j n", n=TPC) \
                        [64 * (h % 2):64 * (h % 2) + DH, (h // 2) % 2, :]
                    nc.vector.tensor_mul(dst, ctx[0:DH, :], rbc[:])

                def make_tail(nctx_set, rep_):

                    def emit_tail_tb(tb):
                        x_t = lnp.tile([128, D], F32, tag="x", name="x_t")
                        for nch in range(2):
                            ps = psmall.tile([128, 512], F32, tag="ps",
                                             name="o_ps")
                            for ch in range(4):
                                nc.tensor.matmul(
                                    ps[:],
                                    nctx_set[ch][:].rearrange(
                                        "p (j n) -> p j n", n=TPC)
                                    [:, :, tb * 128:(tb + 1) * 128],
                                    w_ap("o")[:, 2 * ch:2 * ch + 2,
                                              nch * 512:(nch + 1) * 512],
                                    start=(ch == 0), stop=(ch == 3),
                                    perf_mode=DR)
                            nc.vector.tensor_add(
                                x_t[:, nch * 512:(nch + 1) * 512], ps[:],
                                xr_t[tb][:, nch * 512:(nch + 1) * 512])
                        stats = lnp.tile([128, 2, 6], F32, tag="stats",
                                         name="stats")
                        for sg in range(2):
                            nc.vector.bn_stats(
                                stats[:, sg, :],
                                x_t[:].rearrange("p (s d) -> p s d", s=2)
                                [:, sg, :])
                        mv = lnp.tile([128, 2], F32, tag="mv", name="mv")
                        nc.vector.bn_aggr(mv[:], stats[:])
                        # istd = rsqrt(var+eps): bit-trick + 2 Newton
                        # iterations on DVE ([128,1] micro-ops) so the
                        # scalar engine stays on the Exp act table
                        vpe = lnp.tile([128, 1], F32, tag="vpe", name="vpe")
                        nc.vector.tensor_scalar_add(vpe[:], mv[:, 1:2],
                                                    eps_t[:, 0:1])
                        yi = lnp.tile([128, 1], I32, tag="yi", name="yi")
                        nc.vector.tensor_scalar(
                            yi[:], vpe[:].bitcast(I32), 1, None,
                            ALU.logical_shift_right)
                        nc.vector.tensor_scalar(
                            yi[:], yi[:], -1, 0x5F3759DF, ALU.mult, ALU.add)
                        istd = lnp.tile([128, 1], F32, tag="istd",
                                        name="istd")
                        t_n = lnp.tile([128, 1], F32, tag="t_n", name="t_n")
                        nc.vector.tensor_scalar_mul(istd[:],
                                                    yi[:].bitcast(F32), 1.0)
                        for _ in range(2):
                            nc.vector.tensor_mul(t_n[:], istd[:], istd[:])
                            nc.vector.tensor_mul(t_n[:], t_n[:], vpe[:])
                            nc.vector.tensor_scalar(t_n[:], t_n[:], -0.5,
                                                    1.5, ALU.mult, ALU.add)
                            nc.vector.tensor_mul(istd[:], istd[:], t_n[:])
                        negmi = lnp.tile([128, 1], F32, tag="negmi",
                                         name="negmi")
                        nc.vector.tensor_scalar_mul(negmi[:], mv[:, 0:1],
                                                    -1.0)
                        xn = lnp.tile([128, D], F32, tag="xn", name="xn")
                        nc.gpsimd.tensor_scalar(
                            xn[:], x_t[:], negmi[:, 0:1], istd[:, 0:1],
                            ALU.add, ALU.mult)
                        xg = lnp.tile([128, D], F32, tag="xg", name="xg")
                        nc.vector.tensor_mul(xg[:, 0:512], xn[:, 0:512],
                                             gb_t["gamma"][:, 0:512])
                        nc.gpsimd.tensor_mul(xg[:, 512:D], xn[:, 512:D],
                                             gb_t["gamma"][:, 512:D])
                        nc.gpsimd.dma_start(out[tb * 128:(tb + 1) * 128, :],
                                            xg[:],
                                            accum_op=mybir.AluOpType.add)
                    return emit_tail_tb

                TAIL_INS = [3, 6, 9, 12]
                pend = None
                for idx, (h, kp) in enumerate(flat):
                    pair, off = divmod(h, 2)
                    off *= DH
                    pool = sc_pools[idx % 2]
                    sc = pool.tile([128, 2 * TPC], F32, tag="sc", name="sc")
                    for j in range(2):
                        kb = 2 * kp + j
                        nc.tensor.matmul(
                            sc[:, j * TPC:(j + 1) * TPC],
                            ktf_t[pair][off:off + DH,
                                        kb * 128:(kb + 1) * 128],
                            qt_t[pair][off:off + DH, :],
                            start=True, stop=True)
                    # this rep's Q-proj chunk rides two heads ahead
                    if kp == 1 and h % 2 == 0 and h // 2 + 1 <= 7:
                        emit_qchunk(h // 2 + 1)
                    # next rep's phase 1, interleaved into late heads
                    if next_steps is not None and idx in INS:
                        next_steps[INS.index(idx)]()
                    # previous rep's out-proj/LN tail, squeezed into the
                    # first heads (PE priority stays with this rep's scores)
                    if tail_pending is not None and idx in TAIL_INS:
                        tail_pending(TAIL_INS.index(idx))
                    es = esp.tile([128, 2 * TPC], F8, tag="es", name="es")
                    nc.scalar.activation(es[:], sc[:], AF.Exp,
                                         bias=med_t[:, h:h + 1],
                                         scale=ESCALE)
                    if pend is not None:
                        ph, pkp, pes = pend
                        emit_ctx(ph, pkp, pes)
                        if pkp == NKP - 1:
                            emit_norm(ph)
                    pend = (h, kp, es)
                ph, pkp, pes = pend
                emit_ctx(ph, pkp, pes)
                emit_norm(ph)
                if next_steps is not None:
                    vf_cur = vf_next

                tail_cur = make_tail(nctx_t, rep)
                if rep + 1 >= reps:
                    # last rep: emit its tail right here
                    for tb in range(NTB):
                        tail_cur(tb)
                tail_pending = tail_cur

            for cm in reversed(pools_cm):
                cm.__exit__(None, None, None)
            psmall_cm.__exit__(None, None, None)

    nc.compile()
    return nc


def _dr8(a):
    """[1024, M] f32 -> DoubleRow fp8 [128, 8*M] (k = j*128 + p)."""
    m = a.shape[1]
    return np.ascontiguousarray(
        a.reshape(8, 128, m).transpose(1, 0, 2).reshape(128, 8 * m)
    ).astype(FP8)


def _make_in_maps(hidden_states, attention_mask, Wq, bq, Wk, bk, Wv, bv,
                  med_bias, Wo, bo, gamma, beta):
    x = np.asarray(hidden_states, np.float32)
    mask = np.asarray(attention_mask, np.float32).reshape(B, S)
    med = np.ascontiguousarray(np.asarray(med_bias, np.float32).reshape(H))
    wq8 = _dr8(np.asarray(Wq, np.float32).T * WS)
    wk8 = _dr8(np.asarray(Wk, np.float32).T * WS)
    wv8 = _dr8(np.asarray(Wv, np.float32).T * WS)
    wo8 = _dr8(np.asarray(Wo, np.float32).T * WS)
    bo = np.asarray(bo, np.float32)

    in_maps = []
    for c in range(NCORES):
        b, g = divmod(c, G)
        tsl = slice(g * TPC, (g + 1) * TPC)
        in_maps.append({
            "xt8": _dr8(np.ascontiguousarray(x[b, tsl, :].T)),
            "xres": (WS * (x[b, tsl, :] + bo[None, :])).astype(np.float32),
            "wq8": wq8,
            "wk8": wk8,
            "wv8": wv8,
            "wo8": wo8,
            "bqv": WS * np.asarray(bq, np.float32),
            "bkv": WS * np.asarray(bk, np.float32),
            "bvrow": (WS * np.asarray(bv, np.float32))
                .reshape(1, D).astype(FP8),
            "medv": med,
            "expm": np.ascontiguousarray(
                np.exp(mask[b, tsl]).reshape(NTB, 128).T
            ).astype(np.float32),
            "gamma": np.asarray(gamma, np.float32),
            "beta": np.asarray(beta, np.float32),
        })
    return in_maps


def kernel(**inputs):
    from concourse.bass_utils import run_bass_kernel_spmd

    if "nc" not in _CACHE:
        _CACHE["nc"] = _build()
    nc = _CACHE["nc"]
    in_maps = _make_in_maps(**inputs)
    res = run_bass_kernel_spmd(nc, in_maps, core_ids=list(range(NCORES)))
    out = np.empty((B, S, D), np.float32)
    for c in range(NCORES):
        b, g = divmod(c, G)
        out[b, g * TPC:(g + 1) * TPC, :] = res.results[c]["out"]
    return out
